# revision 13
# baseline (speedup 1.0000x reference)
"""AnyonicNeuron Trainium2 kernel.

The reference applies 28 sequential 2x2 braiding matrices to rows 0,1 of a
[64, 1048576] complex wavefunction, leaves rows 2..63 untouched, then does a
tiny [64] stochastic readout from the diagonal.

Strategy:
  * Host: compose the 28 braiding matrices into ONE 2x2 complex matrix T
    (they act only on rows 0,1 and are each linear), bake its 8 real
    coefficients into the Bass program as immediates.
  * Device (8 NeuronCores, N-axis sharded, embarrassingly parallel):
    stream the shard through SBUF, interleave (re,im) -> complex64 memory
    layout, with rows 0,1 replaced by the 2x2 complex linear combination.
    This is the memory-bound part: 64MB in + 64MB out per core.
  * Host: assemble per-core f32 [64, 2S] outputs, view as complex64, and
    compute the 64-element sigmoid/threshold readout.
"""

import numpy as np

import concourse.bacc as bacc
import concourse.bass as bass
import concourse.mybir as mybir
from concourse.bass_utils import run_bass_kernel_spmd
from concourse.mybir import AluOpType
from concourse.tile import TileContext

DIM = 64
N = 1048576
N_CORES = 8
S = N // N_CORES            # 131072 columns per core
W = 2048                    # phase-A column-block width (two blocks per tile)
FB = S // 128               # 1024, phase-B free dim per partition
EXCHANGE_STATISTICS = np.pi / 4.0
R_LOW = 5000.0
R_HIGH = 10000.0

# test.py can flip these
TRACE = False
LAST_EXEC_NS = None
LAST_RESULTS = None


def _braid_matrix(braiding_angles: np.ndarray) -> np.ndarray:
    """Compose the 28 sequential 2x2 braiding matrices into one (complex128)."""
    iu, ju = np.triu_indices(8, k=1)
    ang = np.asarray(braiding_angles, dtype=np.float32)[iu, ju]
    phases = (ang * np.float32(EXCHANGE_STATISTICS)).astype(np.float64)
    T = np.eye(2, dtype=np.complex128)
    for ph in phases:
        e = np.exp(1j * ph)
        M = np.array([[1.0, e - 1.0], [np.conj(e) - 1.0, 1.0]], dtype=np.complex128)
        T = M @ T
    return T


def _build(A, B, C, D) -> bass.Bass:
    """Per-core SPMD program: out[64, 2S] f32 interleaved complex.

    new_row0 = A*row0 + B*row1 ; new_row1 = C*row0 + D*row1 ; rows 2..63 copied.
    """
    nc = bacc.Bacc()
    xr = nc.dram_tensor("wf_real", [DIM, S], mybir.dt.float32, kind="ExternalInput")
    xi = nc.dram_tensor("wf_imag", [DIM, S], mybir.dt.float32, kind="ExternalInput")
    out = nc.dram_tensor("out", [DIM, 2 * S], mybir.dt.float32, kind="ExternalOutput")

    with TileContext(nc) as tc:
        # ---- phase B: rows 0,1 (full shard as [128, FB] row tiles) ----
        with tc.tile_pool(name="rows01", bufs=1) as pb:
            # one DMA per plane (ISA allows only 2 sem-waits per instruction,
            # so keep the DMA fan-in small)
            rt = pb.tile([128, 2 * FB], mybir.dt.float32, tag="rt", name="rt")
            it_ = pb.tile([128, 2 * FB], mybir.dt.float32, tag="it", name="it")
            nc.sync.dma_start(
                out=rt[:].rearrange("p (r f) -> p r f", r=2),
                in_=xr[0:2, :].rearrange("r (p f) -> p r f", p=128),
            )
            nc.sync.dma_start(
                out=it_[:].rearrange("p (r f) -> p r f", r=2),
                in_=xi[0:2, :].rearrange("r (p f) -> p r f", p=128),
            )
            srcs = {
                "r0r": rt[:, 0:FB],
                "r1r": rt[:, FB : 2 * FB],
                "r0i": it_[:, 0:FB],
                "r1i": it_[:, FB : 2 * FB],
            }

            # STT (scalar_tensor_tensor) ops have only ONE sync-wait slot in
            # the ISA encoding; plain tensor_scalar has two. Touch each DMA'd
            # plane first with a plain tensor_scalar so every later STT needs
            # at most the same-engine DVE wait.
            def acc_tile(tag):
                return pb.tile([128, FB], mybir.dt.float32, tag=tag, name=tag)

            n0r, n0i = acc_tile("n0r"), acc_tile("n0i")
            n1r, n1i = acc_tile("n1r"), acc_tile("n1i")
            nc.vector.tensor_scalar_mul(n0r[:], srcs["r0r"], float(A.real))
            nc.vector.tensor_scalar_mul(n0i[:], srcs["r0i"], float(A.real))
            nc.vector.tensor_scalar_mul(n1r[:], srcs["r0r"], float(C.real))
            nc.vector.tensor_scalar_mul(n1i[:], srcs["r0i"], float(C.real))

            def accum(acc, src, coef):
                nc.vector.scalar_tensor_tensor(
                    acc[:], src, float(coef), acc[:], AluOpType.mult, AluOpType.add
                )

            accum(n0r, srcs["r0i"], -A.imag)
            accum(n0r, srcs["r1r"], B.real)
            accum(n0r, srcs["r1i"], -B.imag)
            accum(n0i, srcs["r0r"], A.imag)
            accum(n0i, srcs["r1r"], B.imag)
            accum(n0i, srcs["r1i"], B.real)
            accum(n1r, srcs["r0i"], -C.imag)
            accum(n1r, srcs["r1r"], D.real)
            accum(n1r, srcs["r1i"], -D.imag)
            accum(n1i, srcs["r0r"], C.imag)
            accum(n1i, srcs["r1r"], D.imag)
            accum(n1i, srcs["r1i"], D.real)

            # all compute on DVE; out-DMAs ride the ACT HWDGE ring so the two
            # rings (SP=loads, ACT=stores) run concurrently
            for row, (tre, tim) in enumerate(((n0r, n0i), (n1r, n1i))):
                o = pb.tile(
                    [128, 2 * FB], mybir.dt.float32, tag=f"o{row}", name=f"o{row}"
                )
                nc.vector.tensor_copy(o[:, 0 : 2 * FB : 2], tre[:])
                nc.vector.tensor_copy(o[:, 1 : 2 * FB : 2], tim[:])
                nc.scalar.dma_start(
                    out=out[row, :].rearrange("(p f) -> p f", p=128), in_=o[:]
                )

            # ---- phase A: rows 2..63 pass-through with interleave ----
            # [124, W] tiles: two column blocks stacked in the partition dim so
            # each DMA spans ~all 16 SDMA engines. Loads on the SP ring
            # (nc.sync), stores on the ACT ring (nc.scalar), copies on DVE.
            with tc.tile_pool(name="copy", bufs=3) as pa:
                for j in range(S // (2 * W)):
                    c0 = 2 * j * W
                    tre = pa.tile([124, W], mybir.dt.float32, tag="tre", name="tre")
                    tim = pa.tile([124, W], mybir.dt.float32, tag="tim", name="tim")
                    to = pa.tile([124, 2 * W], mybir.dt.float32, tag="to", name="to")
                    for b, (lo, hi) in enumerate(((0, 62), (62, 124))):
                        cb = c0 + b * W
                        nc.sync.dma_start(
                            out=tre[lo:hi, :], in_=xr[2:DIM, cb : cb + W]
                        )
                        nc.sync.dma_start(
                            out=tim[lo:hi, :], in_=xi[2:DIM, cb : cb + W]
                        )
                    nc.vector.tensor_copy(to[:, 0 : 2 * W : 2], tre[:])
                    nc.vector.tensor_copy(to[:, 1 : 2 * W : 2], tim[:])
                    for b, (lo, hi) in enumerate(((0, 62), (62, 124))):
                        cb2 = 2 * (c0 + b * W)
                        nc.scalar.dma_start(
                            out=out[2:DIM, cb2 : cb2 + 2 * W], in_=to[lo:hi, :]
                        )
    nc.compile()
    return nc


def kernel(wf_real, wf_imag, braiding_angles, fusion_weights, rand_vals,
           topological_charge, edge_currents):
    global LAST_EXEC_NS, LAST_RESULTS

    wf_real = np.ascontiguousarray(np.asarray(wf_real, dtype=np.float32))
    wf_imag = np.ascontiguousarray(np.asarray(wf_imag, dtype=np.float32))
    braiding_angles = np.asarray(braiding_angles, dtype=np.float32)
    fusion_weights = np.asarray(fusion_weights, dtype=np.float32)
    rand_vals = np.asarray(rand_vals, dtype=np.float32)

    T = _braid_matrix(braiding_angles)
    A, B, C, D = T[0, 0], T[0, 1], T[1, 0], T[1, 1]

    nc = _build(A, B, C, D)

    in_maps = [
        {
            "wf_real": np.ascontiguousarray(wf_real[:, k * S : (k + 1) * S]),
            "wf_imag": np.ascontiguousarray(wf_imag[:, k * S : (k + 1) * S]),
        }
        for k in range(N_CORES)
    ]

    res = run_bass_kernel_spmd(nc, in_maps, core_ids=list(range(N_CORES)), trace=TRACE)
    LAST_EXEC_NS = res.exec_time_ns
    LAST_RESULTS = res

    full = np.empty((DIM, 2 * N), dtype=np.float32)
    for k in range(N_CORES):
        full[:, k * 2 * S : (k + 1) * 2 * S] = res.results[k]["out"]
    braided = full.view(np.complex64)  # [DIM, N]

    # ---- readout (64 elements, host) ----
    d = np.diagonal(braided).astype(np.complex64)
    proj = np.concatenate([d[1:], np.zeros(1, dtype=np.complex64)])
    weighted = np.abs(proj * fusion_weights.astype(np.complex64)).astype(np.float32)
    probs = (1.0 / (1.0 + np.exp(-weighted))).astype(np.float32)
    resistance = np.where(rand_vals < probs, np.float32(R_LOW), np.float32(R_HIGH))
    output = ((resistance - np.float32(R_LOW)) / np.float32(R_HIGH - R_LOW)).astype(
        np.float32
    )
    return output, braided


# revision 14
# speedup vs baseline: 2.3605x; 2.3605x over previous
"""AnyonicNeuron Trainium2 kernel.

The reference applies 28 sequential 2x2 braiding matrices to rows 0,1 of a
[64, 1048576] complex wavefunction, leaves rows 2..63 untouched, then does a
tiny [64] stochastic readout from the diagonal.

Strategy:
  * Host: compose the 28 braiding matrices into ONE 2x2 complex matrix T
    (they act only on rows 0,1 and are each linear), bake its 8 real
    coefficients into the Bass program as immediates.
  * Device (8 NeuronCores, N-axis sharded, embarrassingly parallel):
    stream the shard through SBUF, interleave (re,im) -> complex64 memory
    layout, with rows 0,1 replaced by the 2x2 complex linear combination.
    This is the memory-bound part: 64MB in + 64MB out per core.
  * Host: assemble per-core f32 [64, 2S] outputs, view as complex64, and
    compute the 64-element sigmoid/threshold readout.
"""

import numpy as np

import concourse.bacc as bacc
import concourse.bass as bass
import concourse.mybir as mybir
from concourse.bass_utils import run_bass_kernel_spmd
from concourse.mybir import AluOpType
from concourse.tile import TileContext

DIM = 64
N = 1048576
N_CORES = 8
S = N // N_CORES            # 131072 columns per core
W = 2048                    # phase-A column-block width (two blocks per tile)
FB = S // 128               # 1024, phase-B free dim per partition
EXCHANGE_STATISTICS = np.pi / 4.0
R_LOW = 5000.0
R_HIGH = 10000.0

# test.py can flip these
TRACE = False
LAST_EXEC_NS = None
LAST_RESULTS = None


def _braid_matrix(braiding_angles: np.ndarray) -> np.ndarray:
    """Compose the 28 sequential 2x2 braiding matrices into one (complex128)."""
    iu, ju = np.triu_indices(8, k=1)
    ang = np.asarray(braiding_angles, dtype=np.float32)[iu, ju]
    phases = (ang * np.float32(EXCHANGE_STATISTICS)).astype(np.float64)
    T = np.eye(2, dtype=np.complex128)
    for ph in phases:
        e = np.exp(1j * ph)
        M = np.array([[1.0, e - 1.0], [np.conj(e) - 1.0, 1.0]], dtype=np.complex128)
        T = M @ T
    return T


def _build(A, B, C, D) -> bass.Bass:
    """Per-core SPMD program: out[64, 2S] f32 interleaved complex.

    new_row0 = A*row0 + B*row1 ; new_row1 = C*row0 + D*row1 ; rows 2..63 copied.
    """
    nc = bacc.Bacc()
    xr = nc.dram_tensor("wf_real", [DIM, S], mybir.dt.float32, kind="ExternalInput")
    xi = nc.dram_tensor("wf_imag", [DIM, S], mybir.dt.float32, kind="ExternalInput")
    out = nc.dram_tensor("out", [DIM, 2 * S], mybir.dt.float32, kind="ExternalOutput")

    with TileContext(nc) as tc:
        # ---- phase B: rows 0,1 (full shard as [128, FB] row tiles) ----
        with tc.tile_pool(name="rows01", bufs=1) as pb:
            # one DMA per plane (ISA allows only 2 sem-waits per instruction,
            # so keep the DMA fan-in small)
            rt = pb.tile([128, 2 * FB], mybir.dt.float32, tag="rt", name="rt")
            it_ = pb.tile([128, 2 * FB], mybir.dt.float32, tag="it", name="it")
            nc.gpsimd.dma_start(
                out=rt[:].rearrange("p (r f) -> p r f", r=2),
                in_=xr[0:2, :].rearrange("r (p f) -> p r f", p=128),
            )
            nc.gpsimd.dma_start(
                out=it_[:].rearrange("p (r f) -> p r f", r=2),
                in_=xi[0:2, :].rearrange("r (p f) -> p r f", p=128),
            )
            srcs = {
                "r0r": rt[:, 0:FB],
                "r1r": rt[:, FB : 2 * FB],
                "r0i": it_[:, 0:FB],
                "r1i": it_[:, FB : 2 * FB],
            }

            # STT (scalar_tensor_tensor) ops have only ONE sync-wait slot in
            # the ISA encoding; plain tensor_scalar has two. Touch each DMA'd
            # plane first with a plain tensor_scalar so every later STT needs
            # at most the same-engine DVE wait.
            def acc_tile(tag):
                return pb.tile([128, FB], mybir.dt.float32, tag=tag, name=tag)

            n0r, n0i = acc_tile("n0r"), acc_tile("n0i")
            n1r, n1i = acc_tile("n1r"), acc_tile("n1i")
            nc.vector.tensor_scalar_mul(n0r[:], srcs["r0r"], float(A.real))
            nc.vector.tensor_scalar_mul(n0i[:], srcs["r0i"], float(A.real))
            nc.vector.tensor_scalar_mul(n1r[:], srcs["r0r"], float(C.real))
            nc.vector.tensor_scalar_mul(n1i[:], srcs["r0i"], float(C.real))

            def accum(acc, src, coef):
                nc.vector.scalar_tensor_tensor(
                    acc[:], src, float(coef), acc[:], AluOpType.mult, AluOpType.add
                )

            accum(n0r, srcs["r0i"], -A.imag)
            accum(n0r, srcs["r1r"], B.real)
            accum(n0r, srcs["r1i"], -B.imag)
            accum(n0i, srcs["r0r"], A.imag)
            accum(n0i, srcs["r1r"], B.imag)
            accum(n0i, srcs["r1i"], B.real)
            accum(n1r, srcs["r0i"], -C.imag)
            accum(n1r, srcs["r1r"], D.real)
            accum(n1r, srcs["r1i"], -D.imag)
            accum(n1i, srcs["r0r"], C.imag)
            accum(n1i, srcs["r1r"], D.imag)
            accum(n1i, srcs["r1i"], D.real)

            # all compute on DVE; out-DMAs ride the ACT HWDGE ring so the two
            # rings (SP=loads, ACT=stores) run concurrently
            for row, (tre, tim) in enumerate(((n0r, n0i), (n1r, n1i))):
                o = pb.tile(
                    [128, 2 * FB], mybir.dt.float32, tag=f"o{row}", name=f"o{row}"
                )
                nc.vector.tensor_copy(o[:, 0 : 2 * FB : 2], tre[:])
                nc.vector.tensor_copy(o[:, 1 : 2 * FB : 2], tim[:])
                nc.gpsimd.dma_start(
                    out=out[row, :].rearrange("(p f) -> p f", p=128), in_=o[:]
                )

            # ---- phase A: rows 2..63 pass-through with interleave ----
            # [124, W] tiles: two column blocks stacked in the partition dim so
            # each DMA spans ~all 16 SDMA engines. Loads on the SP ring
            # (nc.sync), stores on the ACT ring (nc.scalar), copies on DVE.
            with tc.tile_pool(name="copy", bufs=3) as pa:
                for j in range(S // (2 * W)):
                    c0 = 2 * j * W
                    tre = pa.tile([124, W], mybir.dt.float32, tag="tre", name="tre")
                    tim = pa.tile([124, W], mybir.dt.float32, tag="tim", name="tim")
                    to = pa.tile([124, 2 * W], mybir.dt.float32, tag="to", name="to")
                    for b, (lo, hi) in enumerate(((0, 62), (62, 124))):
                        cb = c0 + b * W
                        nc.gpsimd.dma_start(
                            out=tre[lo:hi, :], in_=xr[2:DIM, cb : cb + W]
                        )
                        nc.gpsimd.dma_start(
                            out=tim[lo:hi, :], in_=xi[2:DIM, cb : cb + W]
                        )
                    nc.vector.tensor_copy(to[:, 0 : 2 * W : 2], tre[:])
                    nc.vector.tensor_copy(to[:, 1 : 2 * W : 2], tim[:])
                    for b, (lo, hi) in enumerate(((0, 62), (62, 124))):
                        cb2 = 2 * (c0 + b * W)
                        nc.gpsimd.dma_start(
                            out=out[2:DIM, cb2 : cb2 + 2 * W], in_=to[lo:hi, :]
                        )
    nc.compile()
    return nc


def kernel(wf_real, wf_imag, braiding_angles, fusion_weights, rand_vals,
           topological_charge, edge_currents):
    global LAST_EXEC_NS, LAST_RESULTS

    wf_real = np.ascontiguousarray(np.asarray(wf_real, dtype=np.float32))
    wf_imag = np.ascontiguousarray(np.asarray(wf_imag, dtype=np.float32))
    braiding_angles = np.asarray(braiding_angles, dtype=np.float32)
    fusion_weights = np.asarray(fusion_weights, dtype=np.float32)
    rand_vals = np.asarray(rand_vals, dtype=np.float32)

    T = _braid_matrix(braiding_angles)
    A, B, C, D = T[0, 0], T[0, 1], T[1, 0], T[1, 1]

    nc = _build(A, B, C, D)

    in_maps = [
        {
            "wf_real": np.ascontiguousarray(wf_real[:, k * S : (k + 1) * S]),
            "wf_imag": np.ascontiguousarray(wf_imag[:, k * S : (k + 1) * S]),
        }
        for k in range(N_CORES)
    ]

    res = run_bass_kernel_spmd(nc, in_maps, core_ids=list(range(N_CORES)), trace=TRACE)
    LAST_EXEC_NS = res.exec_time_ns
    LAST_RESULTS = res

    full = np.empty((DIM, 2 * N), dtype=np.float32)
    for k in range(N_CORES):
        full[:, k * 2 * S : (k + 1) * 2 * S] = res.results[k]["out"]
    braided = full.view(np.complex64)  # [DIM, N]

    # ---- readout (64 elements, host) ----
    d = np.diagonal(braided).astype(np.complex64)
    proj = np.concatenate([d[1:], np.zeros(1, dtype=np.complex64)])
    weighted = np.abs(proj * fusion_weights.astype(np.complex64)).astype(np.float32)
    probs = (1.0 / (1.0 + np.exp(-weighted))).astype(np.float32)
    resistance = np.where(rand_vals < probs, np.float32(R_LOW), np.float32(R_HIGH))
    output = ((resistance - np.float32(R_LOW)) / np.float32(R_HIGH - R_LOW)).astype(
        np.float32
    )
    return output, braided


# revision 16
# speedup vs baseline: 3.2062x; 1.3582x over previous
"""AnyonicNeuron Trainium2 kernel.

The reference applies 28 sequential 2x2 braiding matrices to rows 0,1 of a
[64, 1048576] complex wavefunction, leaves rows 2..63 untouched, then does a
tiny [64] stochastic readout from the diagonal.

Strategy:
  * Host: compose the 28 braiding matrices into ONE 2x2 complex matrix T
    (they act only on rows 0,1 and are each linear), bake its 8 real
    coefficients into the Bass program as immediates.
  * Device (8 NeuronCores, N-axis sharded, embarrassingly parallel):
    stream the shard through SBUF, interleave (re,im) -> complex64 memory
    layout, with rows 0,1 replaced by the 2x2 complex linear combination.
    This is the memory-bound part: 64MB in + 64MB out per core.
  * Host: assemble per-core f32 [64, 2S] outputs, view as complex64, and
    compute the 64-element sigmoid/threshold readout.
"""

import numpy as np

import concourse.bacc as bacc
import concourse.bass as bass
import concourse.mybir as mybir
from concourse.bass_utils import run_bass_kernel_spmd
from concourse.mybir import AluOpType
from concourse.tile import TileContext

DIM = 64
N = 1048576
N_CORES = 8
S = N // N_CORES            # 131072 columns per core
W = 4096                    # phase-A column-block width (two blocks per tile)
FB = S // 128               # 1024, phase-B free dim per partition
EXCHANGE_STATISTICS = np.pi / 4.0
R_LOW = 5000.0
R_HIGH = 10000.0

# test.py can flip these
TRACE = False
LAST_EXEC_NS = None
LAST_RESULTS = None


def _braid_matrix(braiding_angles: np.ndarray) -> np.ndarray:
    """Compose the 28 sequential 2x2 braiding matrices into one (complex128)."""
    iu, ju = np.triu_indices(8, k=1)
    ang = np.asarray(braiding_angles, dtype=np.float32)[iu, ju]
    phases = (ang * np.float32(EXCHANGE_STATISTICS)).astype(np.float64)
    T = np.eye(2, dtype=np.complex128)
    for ph in phases:
        e = np.exp(1j * ph)
        M = np.array([[1.0, e - 1.0], [np.conj(e) - 1.0, 1.0]], dtype=np.complex128)
        T = M @ T
    return T


def _build(A, B, C, D) -> bass.Bass:
    """Per-core SPMD program: out[64, 2S] f32 interleaved complex.

    new_row0 = A*row0 + B*row1 ; new_row1 = C*row0 + D*row1 ; rows 2..63 copied.
    """
    nc = bacc.Bacc()
    xr = nc.dram_tensor("wf_real", [DIM, S], mybir.dt.float32, kind="ExternalInput")
    xi = nc.dram_tensor("wf_imag", [DIM, S], mybir.dt.float32, kind="ExternalInput")
    out = nc.dram_tensor("out", [DIM, 2 * S], mybir.dt.float32, kind="ExternalOutput")

    with TileContext(nc) as tc:
        # ---- phase B: rows 0,1 (full shard as [128, FB] row tiles) ----
        with tc.tile_pool(name="rows01", bufs=1) as pb:
            # one DMA per plane (ISA allows only 2 sem-waits per instruction,
            # so keep the DMA fan-in small)
            rt = pb.tile([128, 2 * FB], mybir.dt.float32, tag="rt", name="rt")
            it_ = pb.tile([128, 2 * FB], mybir.dt.float32, tag="it", name="it")
            nc.gpsimd.dma_start(
                out=rt[:].rearrange("p (r f) -> p r f", r=2),
                in_=xr[0:2, :].rearrange("r (p f) -> p r f", p=128),
            )
            nc.gpsimd.dma_start(
                out=it_[:].rearrange("p (r f) -> p r f", r=2),
                in_=xi[0:2, :].rearrange("r (p f) -> p r f", p=128),
            )
            srcs = {
                "r0r": rt[:, 0:FB],
                "r1r": rt[:, FB : 2 * FB],
                "r0i": it_[:, 0:FB],
                "r1i": it_[:, FB : 2 * FB],
            }

            # STT (scalar_tensor_tensor) ops have only ONE sync-wait slot in
            # the ISA encoding; plain tensor_scalar has two. Touch each DMA'd
            # plane first with a plain tensor_scalar so every later STT needs
            # at most the same-engine DVE wait.
            def acc_tile(tag):
                return pb.tile([128, FB], mybir.dt.float32, tag=tag, name=tag)

            n0r, n0i = acc_tile("n0r"), acc_tile("n0i")
            n1r, n1i = acc_tile("n1r"), acc_tile("n1i")
            nc.vector.tensor_scalar_mul(n0r[:], srcs["r0r"], float(A.real))
            nc.vector.tensor_scalar_mul(n0i[:], srcs["r0i"], float(A.real))
            nc.vector.tensor_scalar_mul(n1r[:], srcs["r0r"], float(C.real))
            nc.vector.tensor_scalar_mul(n1i[:], srcs["r0i"], float(C.real))

            def accum(acc, src, coef):
                nc.vector.scalar_tensor_tensor(
                    acc[:], src, float(coef), acc[:], AluOpType.mult, AluOpType.add
                )

            accum(n0r, srcs["r0i"], -A.imag)
            accum(n0r, srcs["r1r"], B.real)
            accum(n0r, srcs["r1i"], -B.imag)
            accum(n0i, srcs["r0r"], A.imag)
            accum(n0i, srcs["r1r"], B.imag)
            accum(n0i, srcs["r1i"], B.real)
            accum(n1r, srcs["r0i"], -C.imag)
            accum(n1r, srcs["r1r"], D.real)
            accum(n1r, srcs["r1i"], -D.imag)
            accum(n1i, srcs["r0r"], C.imag)
            accum(n1i, srcs["r1r"], D.imag)
            accum(n1i, srcs["r1i"], D.real)

            # all compute on DVE; out-DMAs ride the ACT HWDGE ring so the two
            # rings (SP=loads, ACT=stores) run concurrently
            for row, (tre, tim) in enumerate(((n0r, n0i), (n1r, n1i))):
                o = pb.tile(
                    [128, 2 * FB], mybir.dt.float32, tag=f"o{row}", name=f"o{row}"
                )
                nc.vector.tensor_copy(o[:, 0 : 2 * FB : 2], tre[:])
                nc.vector.tensor_copy(o[:, 1 : 2 * FB : 2], tim[:])
                nc.gpsimd.dma_start(
                    out=out[row, :].rearrange("(p f) -> p f", p=128), in_=o[:]
                )

            # ---- phase A: rows 2..63 pass-through with interleave ----
            # [124, W] tiles: two column blocks stacked in the partition dim so
            # each DMA spans ~all 16 SDMA engines. Loads on the SP ring
            # (nc.sync), stores on the ACT ring (nc.scalar), copies on DVE.
            # software-pipelined emission: loads for iteration j+1 are emitted
            # BEFORE stores of iteration j, so the single Q7 (SWDGE) queue
            # never stalls on a store's wait with idle load work pending
            with tc.tile_pool(name="copy", bufs=2) as pa:
                NB = S // (2 * W)
                tiles = {}

                def emit_loads(j):
                    c0 = 2 * j * W
                    tre = pa.tile([124, W], mybir.dt.float32, tag="tre", name="tre")
                    tim = pa.tile([124, W], mybir.dt.float32, tag="tim", name="tim")
                    for b, (lo, hi) in enumerate(((0, 62), (62, 124))):
                        cb = c0 + b * W
                        nc.gpsimd.dma_start(
                            out=tre[lo:hi, :], in_=xr[2:DIM, cb : cb + W]
                        )
                        nc.gpsimd.dma_start(
                            out=tim[lo:hi, :], in_=xi[2:DIM, cb : cb + W]
                        )
                    tiles[j] = (tre, tim)

                def emit_compute_store(j):
                    c0 = 2 * j * W
                    tre, tim = tiles.pop(j)
                    to = pa.tile([124, 2 * W], mybir.dt.float32, tag="to", name="to")
                    nc.vector.tensor_copy(to[:, 0 : 2 * W : 2], tre[:])
                    nc.vector.tensor_copy(to[:, 1 : 2 * W : 2], tim[:])
                    for b, (lo, hi) in enumerate(((0, 62), (62, 124))):
                        cb2 = 2 * (c0 + b * W)
                        nc.gpsimd.dma_start(
                            out=out[2:DIM, cb2 : cb2 + 2 * W], in_=to[lo:hi, :]
                        )

                emit_loads(0)
                for j in range(NB):
                    if j + 1 < NB:
                        emit_loads(j + 1)
                    emit_compute_store(j)
    nc.compile()
    return nc


def kernel(wf_real, wf_imag, braiding_angles, fusion_weights, rand_vals,
           topological_charge, edge_currents):
    global LAST_EXEC_NS, LAST_RESULTS

    wf_real = np.ascontiguousarray(np.asarray(wf_real, dtype=np.float32))
    wf_imag = np.ascontiguousarray(np.asarray(wf_imag, dtype=np.float32))
    braiding_angles = np.asarray(braiding_angles, dtype=np.float32)
    fusion_weights = np.asarray(fusion_weights, dtype=np.float32)
    rand_vals = np.asarray(rand_vals, dtype=np.float32)

    T = _braid_matrix(braiding_angles)
    A, B, C, D = T[0, 0], T[0, 1], T[1, 0], T[1, 1]

    nc = _build(A, B, C, D)

    in_maps = [
        {
            "wf_real": np.ascontiguousarray(wf_real[:, k * S : (k + 1) * S]),
            "wf_imag": np.ascontiguousarray(wf_imag[:, k * S : (k + 1) * S]),
        }
        for k in range(N_CORES)
    ]

    res = run_bass_kernel_spmd(nc, in_maps, core_ids=list(range(N_CORES)), trace=TRACE)
    LAST_EXEC_NS = res.exec_time_ns
    LAST_RESULTS = res

    full = np.empty((DIM, 2 * N), dtype=np.float32)
    for k in range(N_CORES):
        full[:, k * 2 * S : (k + 1) * 2 * S] = res.results[k]["out"]
    braided = full.view(np.complex64)  # [DIM, N]

    # ---- readout (64 elements, host) ----
    d = np.diagonal(braided).astype(np.complex64)
    proj = np.concatenate([d[1:], np.zeros(1, dtype=np.complex64)])
    weighted = np.abs(proj * fusion_weights.astype(np.complex64)).astype(np.float32)
    probs = (1.0 / (1.0 + np.exp(-weighted))).astype(np.float32)
    resistance = np.where(rand_vals < probs, np.float32(R_LOW), np.float32(R_HIGH))
    output = ((resistance - np.float32(R_LOW)) / np.float32(R_HIGH - R_LOW)).astype(
        np.float32
    )
    return output, braided


# revision 22
# speedup vs baseline: 6.1481x; 1.9176x over previous
"""AnyonicNeuron Trainium2 kernel.

The reference applies 28 sequential 2x2 braiding matrices to rows 0,1 of a
[64, 1048576] complex wavefunction, leaves rows 2..63 untouched, then does a
tiny [64] stochastic readout from the diagonal.

Strategy:
  * Host: compose the 28 braiding matrices into ONE 2x2 complex matrix T
    (they act only on rows 0,1 and are each linear), bake its 8 real
    coefficients into the Bass program as immediates.
  * Device (8 NeuronCores, N-axis sharded, embarrassingly parallel):
    stream the shard through SBUF, interleave (re,im) -> complex64 memory
    layout, with rows 0,1 replaced by the 2x2 complex linear combination.
    This is the memory-bound part: 64MB in + 64MB out per core.
  * Host: assemble per-core f32 [64, 2S] outputs, view as complex64, and
    compute the 64-element sigmoid/threshold readout.
"""

import time

import numpy as np

import concourse.bacc as bacc
import concourse.bass as bass
import concourse.mybir as mybir
from concourse.bass_utils import run_bass_kernel_spmd
from concourse.mybir import AluOpType
from concourse.tile import TileContext

DIM = 64
N = 1048576
N_CORES = 8
S = N // N_CORES            # 131072 columns per core
W = 4096                    # phase-A column-block width (two blocks per tile)
FB = S // 128               # 1024, phase-B free dim per partition
EXCHANGE_STATISTICS = np.pi / 4.0
R_LOW = 5000.0
R_HIGH = 10000.0

# test.py can flip these
TRACE = False
LAST_EXEC_NS = None
LAST_RESULTS = None

_PROGRAM_CACHE = {}


def _braid_matrix(braiding_angles: np.ndarray) -> np.ndarray:
    """Compose the 28 sequential 2x2 braiding matrices into one (complex128)."""
    iu, ju = np.triu_indices(8, k=1)
    ang = np.asarray(braiding_angles, dtype=np.float32)[iu, ju]
    phases = (ang * np.float32(EXCHANGE_STATISTICS)).astype(np.float64)
    T = np.eye(2, dtype=np.complex128)
    for ph in phases:
        e = np.exp(1j * ph)
        M = np.array([[1.0, e - 1.0], [np.conj(e) - 1.0, 1.0]], dtype=np.complex128)
        T = M @ T
    return T


def _build(A, B, C, D) -> bass.Bass:
    """Per-core SPMD program: out[64, 2S] f32 interleaved complex.

    new_row0 = A*row0 + B*row1 ; new_row1 = C*row0 + D*row1 ; rows 2..63 copied.
    """
    nc = bacc.Bacc()
    xr = nc.dram_tensor("wf_real", [DIM, S], mybir.dt.float32, kind="ExternalInput")
    xi = nc.dram_tensor("wf_imag", [DIM, S], mybir.dt.float32, kind="ExternalInput")
    out = nc.dram_tensor("out", [DIM, 2 * S], mybir.dt.float32, kind="ExternalOutput")

    with TileContext(nc) as tc:
        # ---- phase B: rows 0,1 (full shard as [128, FB] row tiles) ----
        with tc.tile_pool(name="rows01", bufs=1) as pb:
            # one DMA per plane (ISA allows only 2 sem-waits per instruction,
            # so keep the DMA fan-in small)
            rt = pb.tile([128, 2 * FB], mybir.dt.float32, tag="rt", name="rt")
            it_ = pb.tile([128, 2 * FB], mybir.dt.float32, tag="it", name="it")
            nc.gpsimd.dma_start(
                out=rt[:].rearrange("p (r f) -> p r f", r=2),
                in_=xr[0:2, :].rearrange("r (p f) -> p r f", p=128),
            )
            nc.gpsimd.dma_start(
                out=it_[:].rearrange("p (r f) -> p r f", r=2),
                in_=xi[0:2, :].rearrange("r (p f) -> p r f", p=128),
            )
            srcs = {
                "r0r": rt[:, 0:FB],
                "r1r": rt[:, FB : 2 * FB],
                "r0i": it_[:, 0:FB],
                "r1i": it_[:, FB : 2 * FB],
            }

            # STT (scalar_tensor_tensor) ops have only ONE sync-wait slot in
            # the ISA encoding; plain tensor_scalar has two. Touch each DMA'd
            # plane first with a plain tensor_scalar so every later STT needs
            # at most the same-engine DVE wait.
            def acc_tile(tag):
                return pb.tile([128, FB], mybir.dt.float32, tag=tag, name=tag)

            n0r, n0i = acc_tile("n0r"), acc_tile("n0i")
            n1r, n1i = acc_tile("n1r"), acc_tile("n1i")
            nc.vector.tensor_scalar_mul(n0r[:], srcs["r0r"], float(A.real))
            nc.vector.tensor_scalar_mul(n0i[:], srcs["r0i"], float(A.real))
            nc.vector.tensor_scalar_mul(n1r[:], srcs["r0r"], float(C.real))
            nc.vector.tensor_scalar_mul(n1i[:], srcs["r0i"], float(C.real))

            def accum(acc, src, coef):
                nc.vector.scalar_tensor_tensor(
                    acc[:], src, float(coef), acc[:], AluOpType.mult, AluOpType.add
                )

            accum(n0r, srcs["r0i"], -A.imag)
            accum(n0r, srcs["r1r"], B.real)
            accum(n0r, srcs["r1i"], -B.imag)
            accum(n0i, srcs["r0r"], A.imag)
            accum(n0i, srcs["r1r"], B.imag)
            accum(n0i, srcs["r1i"], B.real)
            accum(n1r, srcs["r0i"], -C.imag)
            accum(n1r, srcs["r1r"], D.real)
            accum(n1r, srcs["r1i"], -D.imag)
            accum(n1i, srcs["r0r"], C.imag)
            accum(n1i, srcs["r1r"], D.imag)
            accum(n1i, srcs["r1i"], D.real)

            # all compute on DVE; out-DMAs ride the ACT HWDGE ring so the two
            # rings (SP=loads, ACT=stores) run concurrently
            for row, (tre, tim) in enumerate(((n0r, n0i), (n1r, n1i))):
                o = pb.tile(
                    [128, 2 * FB], mybir.dt.float32, tag=f"o{row}", name=f"o{row}"
                )
                nc.vector.tensor_copy(o[:, 0 : 2 * FB : 2], tre[:])
                nc.vector.tensor_copy(o[:, 1 : 2 * FB : 2], tim[:])
                nc.gpsimd.dma_start(
                    out=out[row, :].rearrange("(p f) -> p f", p=128), in_=o[:]
                )

            # ---- phase A: rows 2..63 pass-through with interleave ----
            # Row-sequential streaming: partition p holds column chunk p of
            # each row (chunk = FB = S/128 elems), Q rows per tile in the free
            # dim. DMAs sweep DRAM contiguously (4KB/8KB descriptors over all
            # 16 SDMA engines at full rate). Loads for iteration j+1 are
            # emitted BEFORE stores of iteration j so the single Q7 (SWDGE)
            # queue never stalls on a store's wait with load work pending.
            with tc.tile_pool(name="copy", bufs=2) as pa:
                Q = 4
                row_groups = []
                r = 2
                while r < DIM:
                    q = min(Q, DIM - r)
                    row_groups.append((r, q))
                    r += q
                NB = len(row_groups)
                tiles = {}

                def emit_loads(j):
                    r0, q = row_groups[j]
                    tre = pa.tile([128, Q * FB], mybir.dt.float32, tag="tre",
                                  name="tre")
                    tim = pa.tile([128, Q * FB], mybir.dt.float32, tag="tim",
                                  name="tim")
                    for t, dram in ((tre, xr), (tim, xi)):
                        nc.gpsimd.dma_start(
                            out=t[:, 0 : q * FB].rearrange(
                                "p (q f) -> p q f", q=q
                            ),
                            in_=dram[r0 : r0 + q, :].rearrange(
                                "q (p f) -> p q f", p=128
                            ),
                        )
                    tiles[j] = (tre, tim)

                def emit_compute_store(j):
                    r0, q = row_groups[j]
                    tre, tim = tiles.pop(j)
                    to = pa.tile([128, Q * 2 * FB], mybir.dt.float32, tag="to",
                                 name="to")
                    to3 = to[:, 0 : q * 2 * FB].rearrange(
                        "p (q f) -> p q f", q=q
                    )
                    nc.vector.tensor_copy(
                        to3[:, :, 0 : 2 * FB : 2],
                        tre[:, 0 : q * FB].rearrange("p (q f) -> p q f", q=q),
                    )
                    nc.vector.tensor_copy(
                        to3[:, :, 1 : 2 * FB : 2],
                        tim[:, 0 : q * FB].rearrange("p (q f) -> p q f", q=q),
                    )
                    nc.gpsimd.dma_start(
                        out=out[r0 : r0 + q, :].rearrange(
                            "q (p f) -> p q f", p=128
                        ),
                        in_=to3,
                    )

                emit_loads(0)
                for j in range(NB):
                    if j + 1 < NB:
                        emit_loads(j + 1)
                    emit_compute_store(j)
    nc.compile()
    return nc


def kernel(wf_real, wf_imag, braiding_angles, fusion_weights, rand_vals,
           topological_charge, edge_currents):
    global LAST_EXEC_NS, LAST_RESULTS

    wf_real = np.ascontiguousarray(np.asarray(wf_real, dtype=np.float32))
    wf_imag = np.ascontiguousarray(np.asarray(wf_imag, dtype=np.float32))
    braiding_angles = np.asarray(braiding_angles, dtype=np.float32)
    fusion_weights = np.asarray(fusion_weights, dtype=np.float32)
    rand_vals = np.asarray(rand_vals, dtype=np.float32)

    T = _braid_matrix(braiding_angles)
    A, B, C, D = T[0, 0], T[0, 1], T[1, 0], T[1, 1]

    key = T.tobytes()
    nc = _PROGRAM_CACHE.get(key)
    if nc is None:
        nc = _build(A, B, C, D)
        _PROGRAM_CACHE[key] = nc

    in_maps = [
        {
            "wf_real": np.ascontiguousarray(wf_real[:, k * S : (k + 1) * S]),
            "wf_imag": np.ascontiguousarray(wf_imag[:, k * S : (k + 1) * S]),
        }
        for k in range(N_CORES)
    ]

    # the axon-tunneled device occasionally reports a transient
    # NRT_EXEC_UNIT_UNRECOVERABLE on the first execute; a retry recovers it
    res = None
    for attempt in range(3):
        try:
            res = run_bass_kernel_spmd(
                nc, in_maps, core_ids=list(range(N_CORES)), trace=TRACE
            )
            break
        except Exception:
            if attempt == 2:
                raise
            time.sleep(5.0)
    LAST_EXEC_NS = res.exec_time_ns
    LAST_RESULTS = res

    full = np.empty((DIM, 2 * N), dtype=np.float32)
    for k in range(N_CORES):
        full[:, k * 2 * S : (k + 1) * 2 * S] = res.results[k]["out"]
    braided = full.view(np.complex64)  # [DIM, N]

    # ---- readout (64 elements, host) ----
    d = np.diagonal(braided).astype(np.complex64)
    proj = np.concatenate([d[1:], np.zeros(1, dtype=np.complex64)])
    weighted = np.abs(proj * fusion_weights.astype(np.complex64)).astype(np.float32)
    probs = (1.0 / (1.0 + np.exp(-weighted))).astype(np.float32)
    resistance = np.where(rand_vals < probs, np.float32(R_LOW), np.float32(R_HIGH))
    output = ((resistance - np.float32(R_LOW)) / np.float32(R_HIGH - R_LOW)).astype(
        np.float32
    )
    return output, braided


# revision 24
# speedup vs baseline: 7.0918x; 1.1535x over previous
"""AnyonicNeuron Trainium2 kernel.

The reference applies 28 sequential 2x2 braiding matrices to rows 0,1 of a
[64, 1048576] complex wavefunction, leaves rows 2..63 untouched, then does a
tiny [64] stochastic readout from the diagonal.

Strategy:
  * Host: compose the 28 braiding matrices into ONE 2x2 complex matrix T
    (they act only on rows 0,1 and are each linear), bake its 8 real
    coefficients into the Bass program as immediates.
  * Device (8 NeuronCores, N-axis sharded, embarrassingly parallel):
    stream the shard through SBUF, interleave (re,im) -> complex64 memory
    layout, with rows 0,1 replaced by the 2x2 complex linear combination.
    This is the memory-bound part: 64MB in + 64MB out per core.
  * Host: assemble per-core f32 [64, 2S] outputs, view as complex64, and
    compute the 64-element sigmoid/threshold readout.
"""

import time

import numpy as np

import concourse.bacc as bacc
import concourse.bass as bass
import concourse.mybir as mybir
from concourse.bass_utils import run_bass_kernel_spmd
from concourse.mybir import AluOpType
from concourse.tile import TileContext

DIM = 64
N = 1048576
N_CORES = 8
S = N // N_CORES            # 131072 columns per core
W = 4096                    # phase-A column-block width (two blocks per tile)
FB = S // 128               # 1024, phase-B free dim per partition
EXCHANGE_STATISTICS = np.pi / 4.0
R_LOW = 5000.0
R_HIGH = 10000.0

# test.py can flip these
TRACE = False
LAST_EXEC_NS = None
LAST_RESULTS = None

_PROGRAM_CACHE = {}


def _braid_matrix(braiding_angles: np.ndarray) -> np.ndarray:
    """Compose the 28 sequential 2x2 braiding matrices into one (complex128)."""
    iu, ju = np.triu_indices(8, k=1)
    ang = np.asarray(braiding_angles, dtype=np.float32)[iu, ju]
    phases = (ang * np.float32(EXCHANGE_STATISTICS)).astype(np.float64)
    T = np.eye(2, dtype=np.complex128)
    for ph in phases:
        e = np.exp(1j * ph)
        M = np.array([[1.0, e - 1.0], [np.conj(e) - 1.0, 1.0]], dtype=np.complex128)
        T = M @ T
    return T


def _build(A, B, C, D) -> bass.Bass:
    """Per-core SPMD program: out[64, 2S] f32 interleaved complex.

    new_row0 = A*row0 + B*row1 ; new_row1 = C*row0 + D*row1 ; rows 2..63 copied.
    """
    nc = bacc.Bacc()
    xr = nc.dram_tensor("wf_real", [DIM, S], mybir.dt.float32, kind="ExternalInput")
    xi = nc.dram_tensor("wf_imag", [DIM, S], mybir.dt.float32, kind="ExternalInput")
    out = nc.dram_tensor("out", [DIM, 2 * S], mybir.dt.float32, kind="ExternalOutput")

    with TileContext(nc) as tc:
        # ---- phase B: rows 0,1 (full shard as [128, FB] row tiles) ----
        with tc.tile_pool(name="rows01", bufs=1) as pb:
            # one DMA per plane (ISA allows only 2 sem-waits per instruction,
            # so keep the DMA fan-in small)
            rt = pb.tile([128, 2 * FB], mybir.dt.float32, tag="rt", name="rt")
            it_ = pb.tile([128, 2 * FB], mybir.dt.float32, tag="it", name="it")
            nc.gpsimd.dma_start(
                out=rt[:].rearrange("p (r f) -> p r f", r=2),
                in_=xr[0:2, :].rearrange("r (p f) -> p r f", p=128),
            )
            nc.gpsimd.dma_start(
                out=it_[:].rearrange("p (r f) -> p r f", r=2),
                in_=xi[0:2, :].rearrange("r (p f) -> p r f", p=128),
            )
            srcs = {
                "r0r": rt[:, 0:FB],
                "r1r": rt[:, FB : 2 * FB],
                "r0i": it_[:, 0:FB],
                "r1i": it_[:, FB : 2 * FB],
            }

            # STT (scalar_tensor_tensor) ops have only ONE sync-wait slot in
            # the ISA encoding; plain tensor_scalar has two. Touch each DMA'd
            # plane first with a plain tensor_scalar so every later STT needs
            # at most the same-engine DVE wait.
            def acc_tile(tag):
                return pb.tile([128, FB], mybir.dt.float32, tag=tag, name=tag)

            n0r, n0i = acc_tile("n0r"), acc_tile("n0i")
            n1r, n1i = acc_tile("n1r"), acc_tile("n1i")
            nc.vector.tensor_scalar_mul(n0r[:], srcs["r0r"], float(A.real))
            nc.vector.tensor_scalar_mul(n0i[:], srcs["r0i"], float(A.real))
            nc.vector.tensor_scalar_mul(n1r[:], srcs["r0r"], float(C.real))
            nc.vector.tensor_scalar_mul(n1i[:], srcs["r0i"], float(C.real))

            def accum(acc, src, coef):
                nc.vector.scalar_tensor_tensor(
                    acc[:], src, float(coef), acc[:], AluOpType.mult, AluOpType.add
                )

            accum(n0r, srcs["r0i"], -A.imag)
            accum(n0r, srcs["r1r"], B.real)
            accum(n0r, srcs["r1i"], -B.imag)
            accum(n0i, srcs["r0r"], A.imag)
            accum(n0i, srcs["r1r"], B.imag)
            accum(n0i, srcs["r1i"], B.real)
            accum(n1r, srcs["r0i"], -C.imag)
            accum(n1r, srcs["r1r"], D.real)
            accum(n1r, srcs["r1i"], -D.imag)
            accum(n1i, srcs["r0r"], C.imag)
            accum(n1i, srcs["r1r"], D.imag)
            accum(n1i, srcs["r1i"], D.real)

            # emitted from inside the phase-A loop (after iteration 2) so the
            # Q7 queue reaches these stores' DVE-chain wait long after it is
            # already satisfied — an early emission would stall all later
            # load emission behind it
            def emit_pb_stores():
                for row, (tre, tim) in enumerate(((n0r, n0i), (n1r, n1i))):
                    o = pb.tile(
                        [128, 2 * FB], mybir.dt.float32, tag=f"o{row}",
                        name=f"o{row}"
                    )
                    nc.vector.tensor_copy(o[:, 0 : 2 * FB : 2], tre[:])
                    nc.vector.tensor_copy(o[:, 1 : 2 * FB : 2], tim[:])
                    nc.gpsimd.dma_start(
                        out=out[row, :].rearrange("(p f) -> p f", p=128),
                        in_=o[:],
                    )

            # ---- phase A: rows 2..63 pass-through with interleave ----
            # Row-sequential streaming: partition p holds column chunk p of
            # each row (chunk = FB = S/128 elems), Q rows per tile in the free
            # dim. DMAs sweep DRAM contiguously (4KB/8KB descriptors over all
            # 16 SDMA engines at full rate). Loads for iteration j+1 are
            # emitted BEFORE stores of iteration j so the single Q7 (SWDGE)
            # queue never stalls on a store's wait with load work pending.
            with tc.tile_pool(name="copy", bufs=2) as pa:
                Q = 4
                row_groups = []
                r = 2
                while r < DIM:
                    q = min(Q, DIM - r)
                    row_groups.append((r, q))
                    r += q
                NB = len(row_groups)
                tiles = {}

                def emit_loads(j):
                    r0, q = row_groups[j]
                    tre = pa.tile([128, Q * FB], mybir.dt.float32, tag="tre",
                                  name="tre")
                    tim = pa.tile([128, Q * FB], mybir.dt.float32, tag="tim",
                                  name="tim")
                    for t, dram in ((tre, xr), (tim, xi)):
                        nc.gpsimd.dma_start(
                            out=t[:, 0 : q * FB].rearrange(
                                "p (q f) -> p q f", q=q
                            ),
                            in_=dram[r0 : r0 + q, :].rearrange(
                                "q (p f) -> p q f", p=128
                            ),
                        )
                    tiles[j] = (tre, tim)

                def emit_compute_store(j):
                    r0, q = row_groups[j]
                    tre, tim = tiles.pop(j)
                    to = pa.tile([128, Q * 2 * FB], mybir.dt.float32, tag="to",
                                 name="to")
                    to3 = to[:, 0 : q * 2 * FB].rearrange(
                        "p (q f) -> p q f", q=q
                    )
                    nc.vector.tensor_copy(
                        to3[:, :, 0 : 2 * FB : 2],
                        tre[:, 0 : q * FB].rearrange("p (q f) -> p q f", q=q),
                    )
                    nc.vector.tensor_copy(
                        to3[:, :, 1 : 2 * FB : 2],
                        tim[:, 0 : q * FB].rearrange("p (q f) -> p q f", q=q),
                    )
                    nc.gpsimd.dma_start(
                        out=out[r0 : r0 + q, :].rearrange(
                            "q (p f) -> p q f", p=128
                        ),
                        in_=to3,
                    )

                emit_loads(0)
                for j in range(NB):
                    if j + 1 < NB:
                        emit_loads(j + 1)
                    emit_compute_store(j)
                    if j == 2:
                        emit_pb_stores()
    nc.compile()
    return nc


def kernel(wf_real, wf_imag, braiding_angles, fusion_weights, rand_vals,
           topological_charge, edge_currents):
    global LAST_EXEC_NS, LAST_RESULTS

    wf_real = np.ascontiguousarray(np.asarray(wf_real, dtype=np.float32))
    wf_imag = np.ascontiguousarray(np.asarray(wf_imag, dtype=np.float32))
    braiding_angles = np.asarray(braiding_angles, dtype=np.float32)
    fusion_weights = np.asarray(fusion_weights, dtype=np.float32)
    rand_vals = np.asarray(rand_vals, dtype=np.float32)

    T = _braid_matrix(braiding_angles)
    A, B, C, D = T[0, 0], T[0, 1], T[1, 0], T[1, 1]

    key = T.tobytes()
    nc = _PROGRAM_CACHE.get(key)
    if nc is None:
        nc = _build(A, B, C, D)
        _PROGRAM_CACHE[key] = nc

    in_maps = [
        {
            "wf_real": np.ascontiguousarray(wf_real[:, k * S : (k + 1) * S]),
            "wf_imag": np.ascontiguousarray(wf_imag[:, k * S : (k + 1) * S]),
        }
        for k in range(N_CORES)
    ]

    # the axon-tunneled device occasionally reports a transient
    # NRT_EXEC_UNIT_UNRECOVERABLE on the first execute; a retry recovers it
    res = None
    for attempt in range(3):
        try:
            res = run_bass_kernel_spmd(
                nc, in_maps, core_ids=list(range(N_CORES)), trace=TRACE
            )
            break
        except Exception:
            if attempt == 2:
                raise
            time.sleep(5.0)
    LAST_EXEC_NS = res.exec_time_ns
    LAST_RESULTS = res

    full = np.empty((DIM, 2 * N), dtype=np.float32)
    for k in range(N_CORES):
        full[:, k * 2 * S : (k + 1) * 2 * S] = res.results[k]["out"]
    braided = full.view(np.complex64)  # [DIM, N]

    # ---- readout (64 elements, host) ----
    d = np.diagonal(braided).astype(np.complex64)
    proj = np.concatenate([d[1:], np.zeros(1, dtype=np.complex64)])
    weighted = np.abs(proj * fusion_weights.astype(np.complex64)).astype(np.float32)
    probs = (1.0 / (1.0 + np.exp(-weighted))).astype(np.float32)
    resistance = np.where(rand_vals < probs, np.float32(R_LOW), np.float32(R_HIGH))
    output = ((resistance - np.float32(R_LOW)) / np.float32(R_HIGH - R_LOW)).astype(
        np.float32
    )
    return output, braided
